# revision 43
# baseline (speedup 1.0000x reference)
"""AdaptiveGridMerger Trainium2 kernel.

Math: the reference scatters x[b,c,:] into a flat 8x8 grid with bilinear
(4-corner) weights from positions[b,c,:], then matmuls grid_weights
GW [270,64].  Equivalent form, per batch:
  S in R[64,306]: column c = wy (x) wx (bilinear hat functions)
  gv  = S @ x          (mm1)
  out = GW @ gv        (mm2)

Sharding: data-parallel over batch, 2 batches per core. Host prep:
x cast to bf16 and pre-permuted into phase-major SBUF-native layout
(xa4/xb4) so every input DMA is fully contiguous (8KB descriptors);
positions pre-packed to one [128,12] tile; GW.T duplicated onto
partitions 64-127 and zero-padded to [128,320] (gw2). The device
output (outd/outt) is likewise in SBUF-native layout and un-permuted
on the host after gather.

Per-core structure (both batches processed together):
 - mm1 packs b0/b1 as two 128x64 column tiles of the PE array: b0's
   gv lands in PSUM partitions 0-63, b1's in 64-127; one 512-col
   stream window feeds both concurrently.
 - mm2 runs in 64x64 4-tile mode: tiles (0,0)/(0,64)/(64,0)/(64,64)
   compute b0/b1 x m-lo/m-hi at once. The 14-row output tail is a
   zero-padded third m-window whose b0/b1 results share one PSUM tile
   (partitions 0-13 / 64-77), written by two matmuls separated in the
   stream to respect the PSUM bank-concurrency rule.
 - T is processed in 8 steps of 512 cols with mm2 lagging mm1 by one
   step, so every PSUM->SBUF copy has a full step of runway; copies
   alternate DVE/ACT. The tail-pair PSUM tile is allocated mid-step so
   its buffer recycling never stalls the next step's first matmul.
 - Input DMAs alternate across both HWDGE rings (sync + scalar);
   per-phase output DMAs go on sync behind the input queue, with the
   last phase split into quarter transfers on two queues.
 - A short spin-matmul burst pre-warms the PE clock (HAM) and filler
   matmuls bridge input-starved steps to keep it warm.
"""

import numpy as np

import concourse.bass as bass
import concourse.bacc as bacc
import concourse.mybir as mybir
from concourse import tile
from concourse.bass_utils import run_bass_kernel_spmd

B, C, T = 16, 306, 4096
M, G, GS = 270, 64, 8
N_CORES = 8
BL = B // N_CORES  # batches per core

NPH = 4            # T phases
TPH = T // NPH     # 1024 cols per phase
TB = 512           # psum window cols
NTB = TPH // TB    # 2 windows per phase

CI = [(0, 128), (128, 128), (256, 50)]   # contraction chunks of C
MW = [(0, 128), (128, 128), (256, 64)]   # m windows (tail padded 14->64)
GW_COLS = 320                             # 270 padded to 64-multiple-ish

N_SPIN = 10

MM_DTYPE = mybir.dt.bfloat16
NP_MM = mybir.dt.np(MM_DTYPE)
FP32 = mybir.dt.float32
OP = mybir.AluOpType


def build_nc():
    nc = bacc.Bacc()
    xa_ext = nc.declare_dram_parameter(
        "xa4", [NPH, 128, 2 * BL * TPH], MM_DTYPE, isOutput=False
    )
    xb_ext = nc.declare_dram_parameter(
        "xb4", [NPH, 50, BL * TPH], MM_DTYPE, isOutput=False
    )
    pos_ext = nc.declare_dram_parameter("posp", [128, 12], FP32, isOutput=False)
    gw_ext = nc.declare_dram_parameter("gw2", [128, GW_COLS], MM_DTYPE, isOutput=False)
    out_ext = nc.declare_dram_parameter(
        "outd", [BL, NPH, 128, 2 * TPH], MM_DTYPE, isOutput=True
    )
    outt_ext = nc.declare_dram_parameter("outt", [BL, 14, T], MM_DTYPE, isOutput=True)

    with tile.TileContext(nc) as tc:
        with (
            tc.tile_pool(name="const", bufs=1) as constp,
            tc.tile_pool(name="xp", bufs=1) as xp,
            tc.tile_pool(name="op", bufs=1) as outp,
            tc.tile_pool(name="gvsb", bufs=3) as gvsbp,
            tc.tile_pool(name="ps2", bufs=8, space=bass.MemorySpace.PSUM) as ps2p,
        ):
            # ---- PE pre-ramp spins (128x64 mode, same as mm1 ci0)
            dummy = constp.tile([128, TB], MM_DTYPE, tag="dummy")
            nc.vector.memset(dummy[:], 0.0)
            spin_ps = ps2p.tile([128, TB], FP32, tag="pb", name="spin_ps")
            for _ in range(N_SPIN):
                nc.tensor.matmul(
                    spin_ps[0:64, :], dummy[:, :64], dummy[:],
                    start=True, stop=True, skip_group_check=True,
                )

            # ---- input DMAs (sync / HWDGE): pos, gw, then x per phase
            # pos layout cols: (b, ci<2): 4b+2ci+d ; ci2: 8+2b+d
            pos_all = constp.tile([128, 12], FP32, tag="pos_all")
            nc.sync.dma_start(out=pos_all[:], in_=pos_ext[:])
            # x tiles: groups (ph0: 1024 cols, ph1: 1024, ph2+3: 2048).
            # xA [128, (ci2)(b2)(tw)] and xB [50, (b2)(tw)] per group.
            gw2 = constp.tile([128, GW_COLS], MM_DTYPE, tag="gw2")
            xAg = {}
            xBg = {}
            # phase-major host layout: each phase's xA is one fully
            # contiguous [128, 8KB/partition] DMA (max descriptor size);
            # xA and xB alternate across the two HWDGE rings.
            for ph in range(NPH):
                xa = xp.tile([128, 2 * BL * TPH], MM_DTYPE, tag=f"xA{ph}", name=f"xA{ph}")
                xb = xp.tile([50, BL * TPH], MM_DTYPE, tag=f"xB{ph}", name=f"xB{ph}")
                ea = nc.sync if ph % 2 == 0 else nc.scalar
                eb = nc.scalar if ph % 2 == 0 else nc.sync
                ea.dma_start(out=xa[:], in_=xa_ext[ph])
                eb.dma_start(out=xb[:], in_=xb_ext[ph])
                if ph == 0:
                    nc.scalar.dma_start(out=gw2[:], in_=gw_ext[:])
                xAg[ph] = xa
                xBg[ph] = xb

            def x_views(ph):
                xav = xAg[ph][:].rearrange("p (ci b t) -> p ci b t", ci=2, b=BL)
                xbv = xBg[ph][:].rearrange("p (b t) -> p b t", b=BL)
                return xav, xbv, 0

            # ---- iota row [0..7]
            io_g = constp.tile([128, GS], FP32, tag="io_g")
            nc.gpsimd.iota(
                io_g[:],
                pattern=[[1, GS]],
                base=0,
                channel_multiplier=0,
                allow_small_or_imprecise_dtypes=True,
            )
            io = constp.tile([128, GS], FP32, tag="io")
            nc.vector.tensor_copy(io[:], io_g[:])

            # ---- hat weights on DVE: w = max(0, min(1-(io-gp), 1+(io-gp)))
            gp = constp.tile([128, 12], FP32, tag="gp")
            nc.vector.tensor_scalar(gp[:], pos_all[:], 1.0, GS / 2.0, OP.add, OP.mult)
            d3 = constp.tile([128, 96], FP32, tag="d3")
            d3v = d3[:].rearrange("p (k j) -> p k j", k=12)
            nc.vector.tensor_tensor(
                d3v,
                io[:].unsqueeze(1).broadcast_to((128, 12, GS)),
                gp[:].unsqueeze(2).broadcast_to((128, 12, GS)),
                OP.subtract,
            )
            m1 = constp.tile([128, 96], FP32, tag="m1")
            nc.vector.tensor_scalar(m1[:], d3[:], -1.0, 1.0, OP.mult, OP.add)
            m2 = constp.tile([128, 96], FP32, tag="m2")
            nc.vector.tensor_scalar(m2[:], d3[:], 1.0, None, OP.add)
            mn3 = constp.tile([128, 96], FP32, tag="mn3")
            nc.vector.tensor_tensor(mn3[:], m1[:], m2[:], OP.min)
            w_all = constp.tile([128, 96], FP32, tag="w_all")
            nc.vector.tensor_scalar(w_all[:], mn3[:], 0.0, None, OP.max)

            # ---- st build: st[(b,ci)][c, 64] = wy (x) wx  (6 outer products)
            st_all = constp.tile([128, 6 * G], MM_DTYPE, tag="st_all")
            wv = w_all[:].rearrange("p (k j) -> p k j", k=12)

            def pos_col(b, ci, d):
                return (4 * b + 2 * ci + d) if ci < 2 else (8 + 2 * b + d)

            for ci in range(3):
                for b in range(BL):
                    k = b * 3 + ci
                    wy = wv[:, pos_col(b, ci, 0), :]
                    wx = wv[:, pos_col(b, ci, 1), :]
                    nc.vector.tensor_tensor(
                        st_all[:, k * G : (k + 1) * G].rearrange(
                            "p (i j) -> p i j", i=GS
                        ),
                        wy.unsqueeze(2).broadcast_to((128, GS, GS)),
                        wx.unsqueeze(1).broadcast_to((128, GS, GS)),
                        OP.mult,
                    )

            def st_sl(b, ci):
                k = b * 3 + ci
                cn = CI[ci][1]
                return st_all[:cn, k * G : (k + 1) * G]

            # ---- persistent output staging tiles
            outch = {}
            for b in range(BL):
                for ph in range(NPH):
                    outch[(b, ph)] = outp.tile(
                        [128, 2 * TPH], MM_DTYPE, tag=f"oc{b}_{ph}", name=f"oc{b}_{ph}"
                    )
            # tail staging: b0 rows at partitions 0-13, b1 rows at 64-77
            stage_pair = outp.tile([128, T], MM_DTYPE, tag="stgp", name="stgp")

            # ---- main pipeline: 8 steps of 512 cols, mm2 lags mm1 by one
            # step so every PSUM->SBUF copy has a full step of runway.
            k_copy = [0]

            def copy_any(dst, src):
                if k_copy[0] % 2 == 0:
                    nc.vector.tensor_copy(dst, src)
                else:
                    nc.scalar.copy(dst, src)
                k_copy[0] += 1

            NSTEP = T // TB  # 8
            N_FILL = {1: 4, 2: 4, 4: 4}
            gv_sbs = {}

            def emit_mm1(k):
                ph = k // NTB
                tb = k % NTB
                xav, xbv, toff = x_views(ph)
                ts = toff + tb * TB
                gv_ps = ps2p.tile([128, TB], FP32, tag="pb", name=f"gv{k}")
                # HAM keep-warm filler: runs while waiting for x DMA, result
                # overwritten by the ci0 start=True matmul below.
                for _ in range(N_FILL.get(k, 0)):
                    nc.tensor.matmul(
                        gv_ps[0:64, :], dummy[:, :64], dummy[:],
                        start=True, stop=True, skip_group_check=True,
                    )
                for ci in range(3):
                    cn = CI[ci][1]
                    for b in range(BL):
                        if ci < 2:
                            rhs = xav[:cn, ci, b, ts : ts + TB]
                        else:
                            rhs = xbv[:cn, b, ts : ts + TB]
                        nc.tensor.matmul(
                            gv_ps[b * 64 : b * 64 + 64, :],
                            st_sl(b, ci),
                            rhs,
                            start=(ci == 0),
                            stop=(ci == 2),
                            skip_group_check=True,
                        )
                gv_sb = gvsbp.tile([128, TB], MM_DTYPE, tag="gvsb", name=f"gvsb{k}")
                copy_any(gv_sb[:], gv_ps[:])
                gv_sbs[k] = gv_sb

            def emit_mm2(k):
                ph = k // NTB
                tb = k % NTB
                t0 = ph * TPH
                ts = tb * TB
                gv_sb = gv_sbs[k]
                # tail pair tile: b0 -> parts 0-63 (tile 0,0), b1 -> parts
                # 64-127 (tile 64,64); the two MMs are separated by mw1's
                # windows so they never touch the bank concurrently. o_t is
                # allocated mid-step (after mw0) so its buffer recycling
                # does not stall the next step's first matmul.
                o_t = None
                m0t, mnt = MW[2]
                for mw, (m0, mn) in enumerate(MW[:2]):
                    o_a = ps2p.tile([128, TB], FP32, tag="pb", name=f"oA{k}_{mw}")
                    o_b = ps2p.tile([128, TB], FP32, tag="pb", name=f"oB{k}_{mw}")
                    for b, o_ps in ((0, o_a), (1, o_b)):
                        rhs = gv_sb[b * 64 : b * 64 + 64, :]
                        for h in range(mn // 64):
                            nc.tensor.matmul(
                                o_ps[h * 64 : h * 64 + 64, :],
                                gw2[b * 64 : b * 64 + 64, m0 + h * 64 : m0 + (h + 1) * 64],
                                rhs,
                                start=True,
                                stop=True,
                                skip_group_check=True,
                            )
                    for b, o_ps in ((0, o_a), (1, o_b)):
                        copy_any(
                            outch[(b, ph)][:, mw * TPH + ts : mw * TPH + ts + TB],
                            o_ps[:],
                        )
                    if mw == 0:
                        o_t = ps2p.tile([128, TB], FP32, tag="pb", name=f"oT{k}")
                        nc.tensor.matmul(
                            o_t[0:64, :],
                            gw2[0:64, m0t : m0t + 64],
                            gv_sb[0:64, :],
                            start=True,
                            stop=True,
                            skip_group_check=True,
                        )
                nc.tensor.matmul(
                    o_t[64:128, :],
                    gw2[64:128, m0t : m0t + 64],
                    gv_sb[64:128, :],
                    start=True,
                    stop=True,
                    skip_group_check=True,
                )
                copy_any(stage_pair[:, t0 + ts : t0 + ts + TB], o_t[:])

            def emit_dmas(k):
                ph = k // NTB
                tb = k % NTB
                t0 = ph * TPH
                ts = tb * TB
                if ph == NPH - 1:
                    # last phase: per-step chunks, issued on two queues in parallel
                    for b in range(BL):
                        eng = nc.sync if b == 0 else nc.scalar
                        eng.dma_start(
                            out=out_ext[b, ph].rearrange("p (mi t) -> p mi t", mi=2)[
                                :, :, ts : ts + TB
                            ],
                            in_=outch[(b, ph)][:].rearrange("p (mi t) -> p mi t", mi=2)[
                                :, :, ts : ts + TB
                            ],
                        )
                elif tb == NTB - 1:
                    # one fully contiguous 0.5 MB transfer per (b, phase)
                    for b in range(BL):
                        nc.sync.dma_start(
                            out=out_ext[b, ph], in_=outch[(b, ph)][:]
                        )
                if tb == NTB - 1:
                    for b in range(BL):
                        nc.sync.dma_start(
                            out=outt_ext[b, :, t0 : t0 + TPH],
                            in_=stage_pair[64 * b : 64 * b + 14, t0 : t0 + TPH],
                        )

            emit_mm1(0)
            for k in range(1, NSTEP):
                emit_mm1(k)
                emit_mm2(k - 1)
                emit_dmas(k - 1)
            emit_mm2(NSTEP - 1)
            emit_dmas(NSTEP - 1)
    nc.compile()
    return nc


def make_in_maps(x, positions, grid_weights):
    gw_t = np.ascontiguousarray(grid_weights.T).astype(NP_MM)  # [64, 270]
    gw2 = np.zeros((128, GW_COLS), dtype=NP_MM)
    gw2[0:64, 0:M] = gw_t
    gw2[64:128, 0:M] = gw_t
    in_maps = []
    for i in range(N_CORES):
        sl = slice(i * BL, (i + 1) * BL)
        xr = x[sl].astype(NP_MM)  # [BL, C, T]
        # xa4[ph, p, ci*2048 + b*1024 + t] = x[b, ci*128+p, ph*1024+t]
        xa4 = np.ascontiguousarray(
            xr[:, 0:256, :]
            .reshape(BL, 2, 128, NPH, TPH)
            .transpose(3, 2, 1, 0, 4)
            .reshape(NPH, 128, 2 * BL * TPH)
        )
        # xb4[ph, p, b*1024 + t] = x[b, 256+p, ph*1024+t]
        xb4 = np.ascontiguousarray(
            xr[:, 256:306, :]
            .reshape(BL, 50, NPH, TPH)
            .transpose(2, 1, 0, 3)
            .reshape(NPH, 50, BL * TPH)
        )
        ps = positions[sl]  # [BL, C, 2]
        posp = np.zeros((128, 12), dtype=np.float32)
        for b in range(BL):
            for ci in range(2):
                posp[:, 4 * b + 2 * ci : 4 * b + 2 * ci + 2] = ps[
                    b, ci * 128 : (ci + 1) * 128, :
                ]
            posp[:50, 8 + 2 * b : 10 + 2 * b] = ps[b, 256:306, :]
        in_maps.append({"xa4": xa4, "xb4": xb4, "posp": posp, "gw2": gw2})
    return in_maps


_NC_CACHE = None


def kernel(x, positions, grid_weights):
    global _NC_CACHE
    if _NC_CACHE is None:
        _NC_CACHE = build_nc()
    nc = _NC_CACHE
    in_maps = make_in_maps(x, positions, grid_weights)
    res = run_bass_kernel_spmd(nc, in_maps, core_ids=list(range(N_CORES)))
    out = np.empty((B, M, T), dtype=np.float32)
    for i, r in enumerate(res.results):
        sl = slice(i * BL, (i + 1) * BL)
        # outd[b, ph, p, mi*1024+t] -> out[b, mi*128+p, ph*1024+t]
        od = np.asarray(r["outd"], dtype=np.float32)
        out[sl, 0:256, :] = od.reshape(BL, NPH, 128, 2, TPH).transpose(
            0, 3, 2, 1, 4
        ).reshape(BL, 256, T)
        out[sl, 256:270, :] = np.asarray(r["outt"], dtype=np.float32)
    return out


if __name__ == "__main__":
    xs = np.random.randn(B, C, T).astype(np.float32)
    ps = np.random.uniform(-1, 0.74, (B, C, 2)).astype(np.float32)
    gw = np.random.randn(M, G).astype(np.float32)
    out = kernel(xs, ps, gw)
    print(out.shape, out.dtype)
